# revision 44
# baseline (speedup 1.0000x reference)
"""AttentiveGraphConvolution (GAT-style layer) on 8 trn2 NeuronCores.

Math (reference):
    h   = x @ W                       [N, D]
    a_s = h @ attn_self               [N, 1]
    a_n = h @ attn_neigh              [N, 1]
    e   = leaky_relu(a_s + a_n.T, 0.2)
    e   = e + NEG_INF * (1 - adj)
    out = relu(softmax(e, -1) @ h)

Reformulation (exact in fp32 up to rounding):
    exp(leaky(s)) = exp(0.2 s) * max(exp(0.8 s), 1),  s_ij = a_s_i + a_n_j
    q2_ji = adjT_ji * max(w_i, m_j) * v2_j            [j, i] layout
            with w = e^{0.8 a_s}, m = e^{-0.8 a_n}, v2 = e^{a_n}
    out_i = relu( (sum_j q2_ji h_j) / (sum_j q2_ji) )  (u2_i cancels)

Collective-free, single-pass design (v2, trace-driven):
  * No AllGather: every core computes the full h = x @ W from a
    replicated bf16 x (the old CC barrier+gather cost ~80us serialized).
  * Few, big DMAs: x ships as 8 x 1MB block tiles, adj as 16 x 1MB
    GP-interleaved supers.  The first version used 125 small DMAs and
    the sync sequencer's ~0.6us/trigger serialized the whole head of
    the kernel.  The a_n round-trip DMAs ride the idle gpsimd queue.
  * v2 = e^{a_n} folds into the DVE tensor_scalar op as its second
    scalar ((wb max m_j) * v2_j), so h chunks need no per-chunk scaled
    copy: 4 plain transposes batch into one [128,512] ACT copy, and the
    denominator weights become a constant ones column (no LDW churn).
  * Phase 1 interleaves with the main loop per x-block (block b ->
    supers 2b, 2b+1), so the PE never idles >1us and the HAM clock
    gate stays at 2.4 GHz; adj super triggers are emitted inside the
    main loop so the 5-buffer adjacency ring never stalls the sync
    queue.
  * DVE: tensor_scalar(max,mult) [4x mode] + one batched 4096-wide
    tensor_tensor(mult) [2x mode] per super (scalar_tensor_tensor has
    no DVE perf mode - that was the baseline's 82us co-bottleneck).
"""

import numpy as np

N = 8192
DIN = 512
DOUT = 128
NCORES = 8
S = N // NCORES     # 1024 output rows per core
P = 128
JC = N // P         # 64 j chunks
KC = DIN // P       # 4 contraction chunks
XB = 8              # x node blocks of 1024
SUP = 4             # j chunks per DVE/adj super-tile
GN = JC // SUP      # 16 super tiles
GP = 4              # adjacency rows per partition line (DMA descriptor size)
ADJ_BUFS = 6


def _emit(nc, tc, ctx, n, s, din, dout):
    from concourse import masks, mybir

    f32 = mybir.dt.float32
    bf16 = mybir.dt.bfloat16
    AF = mybir.ActivationFunctionType
    ALU = mybir.AluOpType

    adjt = nc.dram_tensor("adjt", [n, s], bf16, kind="ExternalInput")
    xt = nc.dram_tensor("xt", [XB * P, KC * 8 * P], bf16, kind="ExternalInput")
    wmat2 = nc.dram_tensor("wmat2", [P, KC * dout], bf16, kind="ExternalInput")
    att = nc.dram_tensor("att", [dout, 2], bf16, kind="ExternalInput")
    out = nc.dram_tensor("out", [s, dout], f32, kind="ExternalOutput")

    const_pool = ctx.enter_context(tc.tile_pool(name="const", bufs=1))
    ph1_pool = ctx.enter_context(tc.tile_pool(name="ph1", bufs=1))
    adj_pool = ctx.enter_context(tc.tile_pool(name="adj", bufs=ADJ_BUFS))
    t1_pool = ctx.enter_context(tc.tile_pool(name="t1", bufs=2))
    q_pool = ctx.enter_context(tc.tile_pool(name="q", bufs=3))
    fin_pool = ctx.enter_context(tc.tile_pool(name="fin", bufs=2))
    dram_pool = ctx.enter_context(tc.tile_pool(name="dram", bufs=1, space="DRAM"))
    ph1_psum = ctx.enter_context(tc.tile_pool(name="ph1_psum", bufs=2, space="PSUM"))
    tp_psum = ctx.enter_context(tc.tile_pool(name="tp_psum", bufs=2, space="PSUM"))
    acc_psum = ctx.enter_context(tc.tile_pool(name="acc_psum", bufs=1, space="PSUM"))

    ident = const_pool.tile([P, P], f32, name="ident")
    masks.make_identity(nc, ident[:])
    identb = const_pool.tile([P, P], bf16, name="identb")
    nc.scalar.activation(identb[:], ident[:], AF.Copy)
    ones1 = const_pool.tile([1, P], bf16, name="ones1")
    nc.gpsimd.memset(ones1[:], 1.0)
    onescol = const_pool.tile([P, 1], bf16, name="onescol")
    nc.gpsimd.memset(onescol[:], 1.0)

    # ---- input DMAs (big, few) ------------------------------------------
    w_sb = const_pool.tile([P, KC * dout], bf16, name="w_sb")
    nc.sync.dma_start(w_sb[:], wmat2[:])
    att_sb = const_pool.tile([P, 2], bf16, name="att_sb")
    nc.sync.dma_start(att_sb[:], att[:])

    x_pool = ctx.enter_context(tc.tile_pool(name="xp", bufs=3))
    x_sb = {}

    def dma_x_block(b):
        t = x_pool.tile([P, KC * 8 * P], bf16, name="x_sb")
        nc.sync.dma_start(t[:], xt[b * P:(b + 1) * P, :])
        x_sb[b] = t

    adj_t = {}

    def dma_adj_super(g):
        at = adj_pool.tile([P, SUP * s], bf16, name="adj_t")
        nc.sync.dma_start(
            at[:],
            adjt[g * SUP * P:(g + 1) * SUP * P, :].rearrange(
                "(p r) i -> p (r i)", r=GP),
        )
        adj_t[g] = at

    dma_x_block(0)
    dma_adj_super(0)
    dma_x_block(1)
    dma_adj_super(1)
    dma_x_block(2)
    dma_adj_super(2)
    dma_adj_super(3)

    # ---- persistent phase-1 tiles ---------------------------------------
    avn_pool = ctx.enter_context(tc.tile_pool(name="avn", bufs=2))
    hT_sb = ph1_pool.tile([P, n], bf16, name="hT_sb")
    m_sb = ph1_pool.tile([P, JC], f32, name="m_sb")
    ean_sb = ph1_pool.tile([P, JC], f32, name="ean_sb")
    h_sb = ph1_pool.tile([P, n], bf16, name="h_sb")
    wb_sb = ph1_pool.tile([P, s], bf16, name="wb_sb")
    an_dram = dram_pool.tile([JC, P], f32, name="an_dram")

    NB = 512
    NBM = 512
    mm_ps = [acc_psum.tile([P, NBM], f32, name=f"mm_ps{b}") for b in range(2)]
    rs_ps = [acc_psum.tile([1, NBM], f32, name=f"rs_ps{b}") for b in range(2)]

    def emit_transposes(g):
        # h chunks for super g: 4 transposes -> one batched ACT copy
        hn_ps = tp_psum.tile([P, SUP * P], bf16, name="hn_ps", tag="tp")
        for r in range(SUP):
            j = g * SUP + r
            nc.tensor.matmul(hn_ps[:, r * P:(r + 1) * P],
                             hT_sb[:, j * P:(j + 1) * P], identb[:],
                             is_transpose=True, start=True, stop=True)
        c0 = g * SUP * P
        nc.scalar.activation(h_sb[:, c0:c0 + SUP * P], hn_ps[:], AF.Copy)

    def emit_super(g):
        # adjacency ring refill (trigger lands on sync with deps satisfied)
        if 4 <= g + 4 < GN:
            dma_adj_super(g + 4)
        # q2 = (wb max m_j) * ean_j * adjT   (DVE: 4x TS + 2x batched TT)
        t1 = t1_pool.tile([P, SUP * s], bf16, name="t1")
        for r in range(SUP):
            j = g * SUP + r
            nc.vector.tensor_scalar(t1[:, r * s:(r + 1) * s], wb_sb[:],
                                    m_sb[:, j:j + 1], ean_sb[:, j:j + 1],
                                    ALU.max, ALU.mult)
        q_t = q_pool.tile([P, SUP * s], bf16, name="q_t")
        nc.vector.tensor_tensor(q_t[:], t1[:], adj_t[g][:], ALU.mult)
        # accumulate numerator, then denominator (one ones-LDW per super)
        for r in range(SUP):
            j = g * SUP + r
            for b in range(2):
                nc.tensor.matmul(mm_ps[b][:], h_sb[:, j * P:(j + 1) * P],
                                 q_t[:, r * s + b * NBM:r * s + (b + 1) * NBM],
                                 start=(j == 0), stop=(j == JC - 1))
        for r in range(SUP):
            j = g * SUP + r
            for b in range(2):
                nc.tensor.matmul(rs_ps[b][:], onescol[:],
                                 q_t[:, r * s + b * NBM:r * s + (b + 1) * NBM],
                                 start=(j == 0), stop=(j == JC - 1))

    # ---- phase 1 (per x block) interleaved with the main loop -----------
    for b in range(XB):
        if b + 3 < XB:
            dma_x_block(b + 3)
        xb = x_sb[b]
        avn_sb = avn_pool.tile([2, 8 * P], f32, name="avn_sb")
        # hT[:, block] = (x @ W).T
        for half in range(2):
            c0 = b * 8 * P + half * NB
            hT_ps = ph1_psum.tile([P, NB], f32, name="hT_ps", tag="ph1")
            for k in range(KC):
                nc.tensor.matmul(
                    hT_ps[:], w_sb[:, k * dout:(k + 1) * dout],
                    xb[:, k * 8 * P + half * NB:k * 8 * P + (half + 1) * NB],
                    start=(k == 0), stop=(k == KC - 1),
                )
            nc.scalar.activation(hT_sb[:, c0:c0 + NB], hT_ps[:], AF.Copy)
        # h chunks for this block's supers (ready one block ahead of use)
        emit_transposes(2 * b)
        emit_transposes(2 * b + 1)
        # a_s / a_n rows for the block
        for half in range(2):
            c0 = b * 8 * P + half * NB
            av_ps = ph1_psum.tile([2, NB], f32, name="av_ps", tag="ph1")
            nc.tensor.matmul(av_ps[:], att_sb[:], hT_sb[:, c0:c0 + NB],
                             start=True, stop=True)
            nc.scalar.activation(avn_sb[:, half * NB:(half + 1) * NB],
                                 av_ps[:], AF.Copy)
        # a_n -> DRAM round trip launch (gpsimd queue); the transpose+exp
        # land AFTER this block's supers so the PE never waits on it
        nc.gpsimd.dma_start(
            an_dram[b * 8:(b + 1) * 8, :].rearrange("k p -> (k p)")[None, :],
            avn_sb[1:2, :])
        anraw_sb = fin_pool.tile([8, P], f32, name="anraw_sb")
        nc.gpsimd.dma_start(anraw_sb[:], an_dram[b * 8:(b + 1) * 8, :])

        def finish_an(b=b, anraw_sb=anraw_sb):
            anT_ps = tp_psum.tile([P, 8], f32, name="anT_ps", tag="tp")
            nc.tensor.matmul(anT_ps[:], anraw_sb[:], ident[:8, :8],
                             is_transpose=True, start=True, stop=True)
            nc.scalar.activation(m_sb[:, b * 8:(b + 1) * 8], anT_ps[:],
                                 AF.Exp, scale=-0.8)
            nc.scalar.activation(ean_sb[:, b * 8:(b + 1) * 8], anT_ps[:],
                                 AF.Exp, scale=1.0)

        if b == 0:
            # wb[p, i] = exp(0.8 a_s_i) for the local slab (chunks 0..7)
            wrow_sb = ph1_pool.tile([1, s], bf16, name="wrow_sb")
            nc.scalar.activation(wrow_sb[:], avn_sb[0:1, :], AF.Exp, scale=0.8)
            for half in range(2):
                wb_ps = ph1_psum.tile([P, NB], f32, name="wb_ps", tag="ph1")
                nc.tensor.matmul(wb_ps[:], ones1[:],
                                 wrow_sb[:, half * NB:(half + 1) * NB],
                                 start=True, stop=True)
                nc.scalar.activation(wb_sb[:, half * NB:(half + 1) * NB],
                                     wb_ps[:], AF.Copy)
        # supers with a one-super lag: super 2b-1 gets a full block of
        # slack for its a_n round trip, super 2b gets one super's worth
        if b == 0:
            finish_an()
            emit_super(0)
        else:
            emit_super(2 * b - 1)
            finish_an()
            emit_super(2 * b)
    emit_super(GN - 1)

    # ---- tail: normalize, relu, transpose out ----------------------------
    rs_sb = ph1_pool.tile([1, s], f32, name="rs_sb")
    mo_sb = ph1_pool.tile([P, s], bf16, name="mo_sb")
    rsT_ps = tp_psum.tile([P, 8], f32, name="rsT_ps", tag="tp")
    # denominator chain first (it gates everything downstream)
    for b in range(2):
        nc.scalar.activation(rs_sb[:, b * NBM:(b + 1) * NBM], rs_ps[b][:],
                             AF.Copy)
        for c in range(4 * b, 4 * b + 4):
            nc.tensor.matmul(rsT_ps[:, c:c + 1], rs_sb[0:1, c * P:(c + 1) * P],
                             ident[:1, :1], is_transpose=True,
                             start=True, stop=True)
    rrT_sb = ph1_pool.tile([P, 8], f32, name="rrT_sb")
    nc.vector.reciprocal(rrT_sb[:], rsT_ps[:])
    for b in range(2):
        nc.scalar.activation(mo_sb[:, b * NBM:(b + 1) * NBM], mm_ps[b][:],
                             AF.Copy)

    for c in range(8):
        ot_ps = tp_psum.tile([P, P], bf16, name="ot_ps", tag="tp")
        nc.tensor.matmul(ot_ps[:], mo_sb[:, c * P:(c + 1) * P], identb[:],
                         is_transpose=True, start=True, stop=True)
        oc_sb = fin_pool.tile([P, dout], f32, name="oc_sb")
        nc.scalar.activation(oc_sb[:], ot_ps[:], AF.Relu,
                             scale=rrT_sb[:, c:c + 1])
        nc.sync.dma_start(out[c * P:(c + 1) * P, :], oc_sb[:])


def build_nc(n=N, s=S, din=DIN, dout=DOUT):
    from contextlib import ExitStack

    import concourse.bacc as bacc
    import concourse.tile as tile

    nc = bacc.Bacc(
        "TRN2",
        target_bir_lowering=False,
        debug=False,
        num_devices=NCORES,
    )
    with tile.TileContext(nc) as tc, ExitStack() as ctx:
        _emit(nc, tc, ctx, n, s, din, dout)
    nc.compile()
    return nc


def prep_adjt(adj_slab_t):
    """[n, s] transposed adj slab -> GP-row-interleaved layout."""
    n, s = adj_slab_t.shape
    g = n // (GP * P)
    return np.ascontiguousarray(
        adj_slab_t.reshape(g, GP, P, s).transpose(0, 2, 1, 3).reshape(n, s))


def make_in_maps(x, adj, W, attn_self, attn_neigh, s=S):
    import ml_dtypes

    bf = ml_dtypes.bfloat16
    att = np.concatenate([attn_self, attn_neigh], axis=1).astype(bf)
    wmat2 = np.ascontiguousarray(
        np.concatenate([W[k * P:(k + 1) * P, :] for k in range(KC)],
                       axis=1).astype(bf))
    xb = x.astype(bf)
    adjb = adj.astype(bf)
    in_maps = []
    for c in range(NCORES):
        perm = np.concatenate([np.arange(c * s, N), np.arange(0, c * s)])
        adjt = prep_adjt(np.ascontiguousarray(adjb[c * s:(c + 1) * s, :][:, perm].T))
        xt_r = xb[perm, :].T  # [din, n] rotated
        blocks = []
        for b in range(XB):
            blocks.append(np.concatenate(
                [xt_r[k * P:(k + 1) * P, b * 8 * P:(b + 1) * 8 * P]
                 for k in range(KC)], axis=1))
        xt = np.ascontiguousarray(np.concatenate(blocks, axis=0))
        in_maps.append({
            "adjt": adjt,
            "xt": xt,
            "wmat2": wmat2,
            "att": att,
        })
    return in_maps


def kernel(x, adj, W, attn_self, attn_neigh):
    from concourse.bass_utils import run_bass_kernel_spmd

    x = np.asarray(x, dtype=np.float32)
    adj = np.asarray(adj, dtype=np.float32)
    W = np.asarray(W, dtype=np.float32)
    attn_self = np.asarray(attn_self, dtype=np.float32)
    attn_neigh = np.asarray(attn_neigh, dtype=np.float32)

    nc = build_nc()
    in_maps = make_in_maps(x, adj, W, attn_self, attn_neigh)
    res = run_bass_kernel_spmd(nc, in_maps, list(range(NCORES)))
    return np.concatenate([res.results[c]["out"] for c in range(NCORES)], axis=0)


# revision 48
# speedup vs baseline: 1.3508x; 1.3508x over previous
"""AttentiveGraphConvolution (GAT-style layer) on 8 trn2 NeuronCores.

Math (reference):
    h   = x @ W                       [N, D]
    a_s = h @ attn_self               [N, 1]
    a_n = h @ attn_neigh              [N, 1]
    e   = leaky_relu(a_s + a_n.T, 0.2)
    e   = e + NEG_INF * (1 - adj)
    out = relu(softmax(e, -1) @ h)

Reformulation (exact in fp32 up to rounding):
    exp(leaky(s)) = exp(0.2 s) * max(exp(0.8 s), 1),  s_ij = a_s_i + a_n_j
    q2_ji = adjT_ji * max(w_i, m_j) * v2_j            [j, i] layout
            with w = e^{0.8 a_s}, m = e^{-0.8 a_n}, v2 = e^{a_n}
    out_i = relu( (sum_j q2_ji h_j) / (sum_j q2_ji) )  (u2_i cancels)

Collective-free, single-pass design (v2, trace-driven):
  * No AllGather: every core computes the full h = x @ W from a
    replicated bf16 x (the old CC barrier+gather cost ~80us serialized).
  * Few, big DMAs: x ships as 8 x 1MB block tiles, adj as 16 x 1MB
    GP-interleaved supers.  The first version used 125 small DMAs and
    the sync sequencer's ~0.6us/trigger serialized the whole head of
    the kernel.  The a_n round-trip DMAs ride the idle gpsimd queue.
  * v2 = e^{a_n} folds into the DVE tensor_scalar op as its second
    scalar ((wb max m_j) * v2_j), so h chunks need no per-chunk scaled
    copy: 4 plain transposes batch into one [128,512] ACT copy, and the
    denominator weights become a constant ones column (no LDW churn).
  * Phase 1 interleaves with the main loop per x-block (block b ->
    supers 2b, 2b+1), so the PE never idles >1us and the HAM clock
    gate stays at 2.4 GHz; adj super triggers are emitted inside the
    main loop so the 5-buffer adjacency ring never stalls the sync
    queue.
  * DVE: tensor_scalar(max,mult) [4x mode] + one batched 4096-wide
    tensor_tensor(mult) [2x mode] per super (scalar_tensor_tensor has
    no DVE perf mode - that was the baseline's 82us co-bottleneck).
"""

import numpy as np

N = 8192
DIN = 512
DOUT = 128
NCORES = 8
S = N // NCORES     # 1024 output rows per core
P = 128
JC = N // P         # 64 j chunks
KC = DIN // P       # 4 contraction chunks
XB = 8              # x node blocks of 1024
SUP = 4             # j chunks per DVE/adj super-tile
GN = JC // SUP      # 16 super tiles
GP = 4              # adjacency rows per partition line (DMA descriptor size)
ADJ_BUFS = 6


def _emit(nc, tc, ctx, n, s, din, dout):
    from concourse import masks, mybir

    f32 = mybir.dt.float32
    bf16 = mybir.dt.bfloat16
    AF = mybir.ActivationFunctionType
    ALU = mybir.AluOpType

    adjt = nc.dram_tensor("adjt", [n, s], bf16, kind="ExternalInput")
    xt = nc.dram_tensor("xt", [XB * P, KC * 8 * P], bf16, kind="ExternalInput")
    wmat2 = nc.dram_tensor("wmat2", [P, KC * dout], bf16, kind="ExternalInput")
    att = nc.dram_tensor("att", [dout, 2], bf16, kind="ExternalInput")
    out = nc.dram_tensor("out", [s, dout], f32, kind="ExternalOutput")

    const_pool = ctx.enter_context(tc.tile_pool(name="const", bufs=1))
    ph1_pool = ctx.enter_context(tc.tile_pool(name="ph1", bufs=1))
    adj_pool = ctx.enter_context(tc.tile_pool(name="adj", bufs=ADJ_BUFS))
    t1_pool = ctx.enter_context(tc.tile_pool(name="t1", bufs=2))
    q_pool = ctx.enter_context(tc.tile_pool(name="q", bufs=3))
    fin_pool = ctx.enter_context(tc.tile_pool(name="fin", bufs=2))
    ph1_psum = ctx.enter_context(tc.tile_pool(name="ph1_psum", bufs=2, space="PSUM"))
    tp_psum = ctx.enter_context(tc.tile_pool(name="tp_psum", bufs=2, space="PSUM"))
    acc_psum = ctx.enter_context(tc.tile_pool(name="acc_psum", bufs=1, space="PSUM"))

    ident = const_pool.tile([P, P], f32, name="ident")
    masks.make_identity(nc, ident[:])
    identb = const_pool.tile([P, P], bf16, name="identb")
    nc.scalar.activation(identb[:], ident[:], AF.Copy)
    ones1 = const_pool.tile([1, P], bf16, name="ones1")
    nc.gpsimd.memset(ones1[:], 1.0)
    onescol = const_pool.tile([P, 1], bf16, name="onescol")
    nc.gpsimd.memset(onescol[:], 1.0)

    # ---- input DMAs (big, few) ------------------------------------------
    w_sb = const_pool.tile([P, KC * dout], bf16, name="w_sb")
    nc.sync.dma_start(w_sb[:], wmat2[:])
    att_sb = const_pool.tile([P, 2], bf16, name="att_sb")
    nc.sync.dma_start(att_sb[:], att[:])

    x_pool = ctx.enter_context(tc.tile_pool(name="xp", bufs=3))
    x_sb = {}

    def dma_x_block(b):
        t = x_pool.tile([P, KC * 8 * P], bf16, name="x_sb")
        nc.sync.dma_start(t[:], xt[b * P:(b + 1) * P, :])
        x_sb[b] = t

    adj_t = {}

    def dma_adj_super(g):
        at = adj_pool.tile([P, SUP * s], bf16, name="adj_t")
        nc.sync.dma_start(
            at[:],
            adjt[g * SUP * P:(g + 1) * SUP * P, :].rearrange(
                "(p r) i -> p (r i)", r=GP),
        )
        adj_t[g] = at

    dma_x_block(0)
    dma_adj_super(0)
    dma_x_block(1)
    dma_adj_super(1)
    dma_x_block(2)
    dma_adj_super(2)
    dma_adj_super(3)

    # ---- persistent phase-1 tiles ---------------------------------------
    hT_sb = ph1_pool.tile([P, n], bf16, name="hT_sb")
    m_sb = ph1_pool.tile([P, JC], f32, name="m_sb")
    ean_sb = ph1_pool.tile([P, JC], f32, name="ean_sb")
    h_sb = ph1_pool.tile([P, n], bf16, name="h_sb")
    wb_sb = ph1_pool.tile([P, s], bf16, name="wb_sb")

    NB = 512
    NBM = 512
    mm_ps = [acc_psum.tile([P, NBM], f32, name=f"mm_ps{b}") for b in range(2)]
    rs_ps = [acc_psum.tile([1, NBM], f32, name=f"rs_ps{b}") for b in range(2)]

    def emit_transposes(g):
        # h chunks for super g: 4 transposes -> one batched ACT copy
        hn_ps = tp_psum.tile([P, SUP * P], bf16, name="hn_ps", tag="tp")
        for r in range(SUP):
            j = g * SUP + r
            nc.tensor.matmul(hn_ps[:, r * P:(r + 1) * P],
                             hT_sb[:, j * P:(j + 1) * P], identb[:],
                             is_transpose=True, start=True, stop=True)
        c0 = g * SUP * P
        nc.scalar.activation(h_sb[:, c0:c0 + SUP * P], hn_ps[:], AF.Copy)

    def emit_super(g):
        # adjacency ring refill (trigger lands on sync with deps satisfied)
        if 4 <= g + 4 < GN:
            dma_adj_super(g + 4)
        # q2 = (wb max m_j) * ean_j * adjT   (DVE: 4x TS + 2x batched TT)
        t1 = t1_pool.tile([P, SUP * s], bf16, name="t1")
        for r in range(SUP):
            j = g * SUP + r
            nc.vector.tensor_scalar(t1[:, r * s:(r + 1) * s], wb_sb[:],
                                    m_sb[:, j:j + 1], ean_sb[:, j:j + 1],
                                    ALU.max, ALU.mult)
        q_t = q_pool.tile([P, SUP * s], bf16, name="q_t")
        nc.vector.tensor_tensor(q_t[:], t1[:], adj_t[g][:], ALU.mult)
        # accumulate numerator, then denominator (one ones-LDW per super)
        for r in range(SUP):
            j = g * SUP + r
            for b in range(2):
                nc.tensor.matmul(mm_ps[b][:], h_sb[:, j * P:(j + 1) * P],
                                 q_t[:, r * s + b * NBM:r * s + (b + 1) * NBM],
                                 start=(j == 0), stop=(j == JC - 1))
        for r in range(SUP):
            j = g * SUP + r
            for b in range(2):
                nc.tensor.matmul(rs_ps[b][:], onescol[:],
                                 q_t[:, r * s + b * NBM:r * s + (b + 1) * NBM],
                                 start=(j == 0), stop=(j == JC - 1))

    # ---- phase 1 (per x block) interleaved with the main loop -----------
    for b in range(XB):
        if b + 3 < XB:
            dma_x_block(b + 3)
        xb = x_sb[b]
        # hT[:, block] = (x @ W).T
        for half in range(2):
            c0 = b * 8 * P + half * NB
            hT_ps = ph1_psum.tile([P, NB], f32, name="hT_ps", tag="ph1")
            for k in range(KC):
                nc.tensor.matmul(
                    hT_ps[:], w_sb[:, k * dout:(k + 1) * dout],
                    xb[:, k * 8 * P + half * NB:k * 8 * P + (half + 1) * NB],
                    start=(k == 0), stop=(k == KC - 1),
                )
            nc.scalar.activation(hT_sb[:, c0:c0 + NB], hT_ps[:], AF.Copy)
        # a_n chunk columns straight from hT: one [128,1] matmul per chunk
        anT_ps = tp_psum.tile([P, 8], f32, name="anT_ps", tag="tp")
        for c8 in range(8):
            j = b * 8 + c8
            nc.tensor.matmul(anT_ps[:, c8:c8 + 1], hT_sb[:, j * P:(j + 1) * P],
                             att_sb[:, 1:2], start=True, stop=True)
        nc.scalar.activation(m_sb[:, b * 8:(b + 1) * 8], anT_ps[:],
                             AF.Exp, scale=-0.8)
        nc.scalar.activation(ean_sb[:, b * 8:(b + 1) * 8], anT_ps[:],
                             AF.Exp, scale=1.0)
        # h chunks for this block's supers (ready one block ahead of use)
        emit_transposes(2 * b)
        emit_transposes(2 * b + 1)
        if b == 0:
            # wb[p, i] = exp(0.8 a_s_i), a_s via the same column trick
            as_ps = tp_psum.tile([P, 8], f32, name="as_ps", tag="tp")
            for c8 in range(8):
                nc.tensor.matmul(as_ps[:, c8:c8 + 1],
                                 hT_sb[:, c8 * P:(c8 + 1) * P],
                                 att_sb[:, 0:1], start=True, stop=True)
            ascol_sb = ph1_pool.tile([P, 8], bf16, name="ascol_sb")
            nc.scalar.activation(ascol_sb[:], as_ps[:], AF.Copy)
            wrow_ps = tp_psum.tile([1, s], bf16, name="wrow_ps", tag="tp")
            for c8 in range(8):
                nc.tensor.matmul(wrow_ps[0:1, c8 * P:(c8 + 1) * P],
                                 ascol_sb[:, c8:c8 + 1], identb[:],
                                 is_transpose=True, start=True, stop=True)
            wrow_sb = ph1_pool.tile([1, s], bf16, name="wrow_sb")
            nc.scalar.activation(wrow_sb[:], wrow_ps[:], AF.Exp, scale=0.8)
            for half in range(2):
                wb_ps = ph1_psum.tile([P, NB], f32, name="wb_ps", tag="ph1")
                nc.tensor.matmul(wb_ps[:], ones1[:],
                                 wrow_sb[:, half * NB:(half + 1) * NB],
                                 start=True, stop=True)
                nc.scalar.activation(wb_sb[:, half * NB:(half + 1) * NB],
                                     wb_ps[:], AF.Copy)
        # supers with a one-super lag for scheduling slack
        if b == 0:
            emit_super(0)
        else:
            emit_super(2 * b - 1)
            emit_super(2 * b)
    emit_super(GN - 1)

    # ---- tail: normalize, relu, transpose out ----------------------------
    rs_sb = ph1_pool.tile([1, s], f32, name="rs_sb")
    mo_sb = ph1_pool.tile([P, s], bf16, name="mo_sb")
    rsT_ps = tp_psum.tile([P, 8], f32, name="rsT_ps", tag="tp")
    # denominator chain first (it gates everything downstream)
    for b in range(2):
        nc.scalar.activation(rs_sb[:, b * NBM:(b + 1) * NBM], rs_ps[b][:],
                             AF.Copy)
        for c in range(4 * b, 4 * b + 4):
            nc.tensor.matmul(rsT_ps[:, c:c + 1], rs_sb[0:1, c * P:(c + 1) * P],
                             ident[:1, :1], is_transpose=True,
                             start=True, stop=True)
    rrT_sb = ph1_pool.tile([P, 8], f32, name="rrT_sb")
    nc.vector.reciprocal(rrT_sb[:], rsT_ps[:])
    for b in range(2):
        nc.scalar.activation(mo_sb[:, b * NBM:(b + 1) * NBM], mm_ps[b][:],
                             AF.Copy)

    for c in range(8):
        ot_ps = tp_psum.tile([P, P], bf16, name="ot_ps", tag="tp")
        nc.tensor.matmul(ot_ps[:], mo_sb[:, c * P:(c + 1) * P], identb[:],
                         is_transpose=True, start=True, stop=True)
        oc_sb = fin_pool.tile([P, dout], f32, name="oc_sb")
        nc.scalar.activation(oc_sb[:], ot_ps[:], AF.Relu,
                             scale=rrT_sb[:, c:c + 1])
        nc.sync.dma_start(out[c * P:(c + 1) * P, :], oc_sb[:])


def build_nc(n=N, s=S, din=DIN, dout=DOUT):
    from contextlib import ExitStack

    import concourse.bacc as bacc
    import concourse.tile as tile

    nc = bacc.Bacc(
        "TRN2",
        target_bir_lowering=False,
        debug=False,
        num_devices=NCORES,
    )
    with tile.TileContext(nc) as tc, ExitStack() as ctx:
        _emit(nc, tc, ctx, n, s, din, dout)
    nc.compile()
    return nc


def prep_adjt(adj_slab_t):
    """[n, s] transposed adj slab -> GP-row-interleaved layout."""
    n, s = adj_slab_t.shape
    g = n // (GP * P)
    return np.ascontiguousarray(
        adj_slab_t.reshape(g, GP, P, s).transpose(0, 2, 1, 3).reshape(n, s))


def make_in_maps(x, adj, W, attn_self, attn_neigh, s=S):
    import ml_dtypes

    bf = ml_dtypes.bfloat16
    att = np.concatenate([attn_self, attn_neigh], axis=1).astype(bf)
    wmat2 = np.ascontiguousarray(
        np.concatenate([W[k * P:(k + 1) * P, :] for k in range(KC)],
                       axis=1).astype(bf))
    xb = x.astype(bf)
    adjb = adj.astype(bf)
    in_maps = []
    for c in range(NCORES):
        perm = np.concatenate([np.arange(c * s, N), np.arange(0, c * s)])
        adjt = prep_adjt(np.ascontiguousarray(adjb[c * s:(c + 1) * s, :][:, perm].T))
        xt_r = xb[perm, :].T  # [din, n] rotated
        blocks = []
        for b in range(XB):
            blocks.append(np.concatenate(
                [xt_r[k * P:(k + 1) * P, b * 8 * P:(b + 1) * 8 * P]
                 for k in range(KC)], axis=1))
        xt = np.ascontiguousarray(np.concatenate(blocks, axis=0))
        in_maps.append({
            "adjt": adjt,
            "xt": xt,
            "wmat2": wmat2,
            "att": att,
        })
    return in_maps


def kernel(x, adj, W, attn_self, attn_neigh):
    from concourse.bass_utils import run_bass_kernel_spmd

    x = np.asarray(x, dtype=np.float32)
    adj = np.asarray(adj, dtype=np.float32)
    W = np.asarray(W, dtype=np.float32)
    attn_self = np.asarray(attn_self, dtype=np.float32)
    attn_neigh = np.asarray(attn_neigh, dtype=np.float32)

    nc = build_nc()
    in_maps = make_in_maps(x, adj, W, attn_self, attn_neigh)
    res = run_bass_kernel_spmd(nc, in_maps, list(range(NCORES)))
    return np.concatenate([res.results[c]["out"] for c in range(NCORES)], axis=0)
